# revision 2
# baseline (speedup 1.0000x reference)
"""GCN block (2-layer) Trainium2 Bass kernel.

Math (per B*T slice, shared graph):
  t2 = relu(A @ (X @ W1) + b1);  out = sigmoid(A @ t2 @ W2 + b2)
  A = D^-1/2 (Adj + I) D^-1/2  (PyG gcn_norm, counts edge multiplicity)

Device mapping:
  A is applied as dense 128x128 blocks of the integer matrix M = Adj + I
  (exact in fp8e4) via PE matmuls accumulating in PSUM; the D^-1/2 factors
  are folded in on the src side (host, into the X cast) and dst side
  (per-partition scale at the PSUM drain).  Layer order is rearranged as
  L1: (X@W1) then A;  L2: A then @W2 — associativity keeps it exact.

Sharding: each of 8 cores owns 10 of the 80 dst-node blocks (128 nodes
each, N padded 10000->10240) for ALL 24 B*T slices; an AllGather of the
relu'd layer-1 activations runs between the layers.

W-stages use block_diag(W, W) stationaries so two slices share one matmul
at full 128-partition width.
"""
import time

import numpy as np
import ml_dtypes

import concourse.bacc as bacc
import concourse.mybir as mybir
import concourse.tile as tile
from concourse.bass_utils import run_bass_kernel_spmd

N_CORES = 8
N = 10000
NP = 10240            # padded nodes
NB = NP // 128        # 80 node blocks
BPC = NB // N_CORES   # 10 dst blocks per core
B, T, C = 2, 12, 64
S = B * T             # 24 slices
PAIRS = S // 2        # 12 slice pairs
F = S * C             # 1536 free columns (slice-major: s*64+f)
CH = 3                # free chunks
FCH = F // CH         # 512 cols per chunk = 4 pairs

f32 = mybir.dt.float32
bf16 = mybir.dt.bfloat16
fp8 = mybir.dt.float8e4


def build_program(with_collective=True):
    nc = bacc.Bacc("TRN2", target_bir_lowering=False, debug=False,
                   num_devices=N_CORES)

    xt_ext = nc.dram_tensor("XT", [PAIRS, 128, NP], bf16, kind="ExternalInput")
    m_ext = nc.dram_tensor("M", [BPC, NB, 128, 128], fp8, kind="ExternalInput")
    w1_ext = nc.dram_tensor("W1d", [128, 128], bf16, kind="ExternalInput")
    w2_ext = nc.dram_tensor("W2d", [128, 128], bf16, kind="ExternalInput")
    b1_ext = nc.dram_tensor("B1", [128, FCH], f32, kind="ExternalInput")
    b2_ext = nc.dram_tensor("B2", [128, 1], f32, kind="ExternalInput")
    di_ext = nc.dram_tensor("DI", [128, BPC], f32, kind="ExternalInput")
    out_ext = nc.dram_tensor("OUT", [PAIRS, 128, BPC * 128], f32,
                             kind="ExternalOutput")

    with tile.TileContext(nc) as tc:
        with (
            tc.tile_pool(name="consts", bufs=1) as consts,
            tc.tile_pool(name="xt", bufs=2) as pool_xt,
            tc.tile_pool(name="w1dr", bufs=3) as pool_w1dr,
            tc.tile_pool(name="xw", bufs=NB + 2) as pool_xw,
            tc.tile_pool(name="m", bufs=2) as pool_m,
            tc.tile_pool(name="u", bufs=2) as pool_u,
            tc.tile_pool(name="t2", bufs=3) as pool_t2,
            tc.tile_pool(name="s2T", bufs=2) as pool_s2t,
            tc.tile_pool(name="outp", bufs=2) as pool_out,
            tc.tile_pool(name="psum", bufs=2, space="PSUM") as pool_psum,
            tc.tile_pool(name="dram", bufs=1, space="DRAM") as dram,
        ):
            # constants
            w1t = consts.tile([128, 128], bf16, tag="w1")
            nc.sync.dma_start(w1t[:], w1_ext[:])
            w2t = consts.tile([128, 128], bf16, tag="w2")
            nc.sync.dma_start(w2t[:], w2_ext[:])
            b1t = consts.tile([128, FCH], f32, tag="b1")
            nc.sync.dma_start(b1t[:], b1_ext[:])
            b2t = consts.tile([128, 1], f32, tag="b2")
            nc.sync.dma_start(b2t[:], b2_ext[:])
            dit = consts.tile([128, BPC], f32, tag="di")
            nc.sync.dma_start(dit[:], di_ext[:])

            # DRAM intermediates
            xw1T = dram.tile([PAIRS, 128, NP], bf16, tag="xw1T")
            t2_loc = dram.tile([BPC * 128, F], bf16, tag="t2loc")
            if with_collective:
                t2_full = dram.tile([NP, F], bf16, tag="t2full",
                                    addr_space="Shared")
            else:
                t2_full = dram.tile([NP, F], bf16, tag="t2full")
            s2_loc = dram.tile([BPC * 128, F], bf16, tag="s2loc")

            # ---- Phase W1: xw1T[pair] = blockdiag(W1,W1)^T @ XT[pair] ----
            for p in range(PAIRS):
                xt = pool_xt.tile([128, NP], bf16, tag="xt")
                nc.sync.dma_start(xt[:], xt_ext[p])
                for k in range(NP // 2048):
                    ps = pool_psum.tile([128, 2048], f32, tag="mm")
                    for j in range(4):
                        nc.tensor.matmul(
                            ps[:, j * 512:(j + 1) * 512], w1t[:],
                            xt[:, k * 2048 + j * 512:k * 2048 + (j + 1) * 512],
                            start=True, stop=True)
                    dr = pool_w1dr.tile([128, 2048], bf16, tag="w1dr")
                    if k % 2 == 0:
                        nc.vector.tensor_scalar_mul(dr[:], ps[:], 1.0)
                    else:
                        nc.scalar.activation(dr[:], ps[:],
                                             mybir.ActivationFunctionType.Copy)
                    nc.sync.dma_start(
                        xw1T[p][:, k * 2048:(k + 1) * 2048], dr[:])

            # merged row view for transpose loads: [(pair row) node]
            xw1T_rows = xw1T[:].rearrange("a p d -> (a p) d")

            # ---- Phase A1: t2 = dinv*relu(dinv*(M @ xw1) + b1) ----
            for ch in range(CH):
                xwt = []
                for bj in range(NB):
                    t = pool_xw.tile([128, FCH], bf16, tag="xw")
                    nc.sync.dma_start(
                        t[:],
                        xw1T_rows[ch * FCH:(ch + 1) * FCH,
                                  bj * 128:(bj + 1) * 128],
                        transpose=True)
                    xwt.append(t)
                for bi in range(BPC):
                    mrow = pool_m.tile([128, NB, 128], fp8, tag="m")
                    nc.sync.dma_start(
                        mrow[:], m_ext[bi].rearrange("b p d -> p b d"))
                    ps = pool_psum.tile([128, 512], f32, tag="mm")
                    for bj in range(NB):
                        nc.tensor.matmul(ps[:], mrow[:, bj, :], xwt[bj][:],
                                         start=(bj == 0), stop=(bj == NB - 1))
                    u = pool_u.tile([128, FCH], f32, tag="u")
                    nc.vector.scalar_tensor_tensor(
                        u[:], ps[:], dit[:, bi:bi + 1], b1t[:],
                        mybir.AluOpType.mult, mybir.AluOpType.add)
                    t2t = pool_t2.tile([128, FCH], bf16, tag="t2")
                    nc.scalar.activation(t2t[:], u[:],
                                         mybir.ActivationFunctionType.Relu,
                                         scale=dit[:, bi:bi + 1])
                    nc.sync.dma_start(
                        t2_loc[bi * 128:(bi + 1) * 128,
                               ch * FCH:(ch + 1) * FCH], t2t[:])

            # ---- AllGather t2 across the 8 cores ----
            if with_collective:
                nc.gpsimd.collective_compute(
                    "AllGather", mybir.AluOpType.bypass,
                    replica_groups=[list(range(N_CORES))],
                    ins=[t2_loc[:]], outs=[t2_full[:]])
            else:
                # timing stand-in: emulate receive traffic
                for r in range(N_CORES):
                    nc.sync.dma_start(
                        t2_full[r * BPC * 128:(r + 1) * BPC * 128, :],
                        t2_loc[:])

            # ---- Phase A2: s2 = dinv*(M @ t2) ----
            for ch in range(CH):
                t2c = []
                for bj in range(NB):
                    t = pool_xw.tile([128, FCH], bf16, tag="xw")
                    nc.sync.dma_start(
                        t[:], t2_full[bj * 128:(bj + 1) * 128,
                                      ch * FCH:(ch + 1) * FCH])
                    t2c.append(t)
                for bi in range(BPC):
                    mrow = pool_m.tile([128, NB, 128], fp8, tag="m")
                    nc.sync.dma_start(
                        mrow[:], m_ext[bi].rearrange("b p d -> p b d"))
                    ps = pool_psum.tile([128, 512], f32, tag="mm")
                    for bj in range(NB):
                        nc.tensor.matmul(ps[:], mrow[:, bj, :], t2c[bj][:],
                                         start=(bj == 0), stop=(bj == NB - 1))
                    s2t = pool_t2.tile([128, FCH], bf16, tag="t2")
                    nc.vector.tensor_scalar_mul(s2t[:], ps[:],
                                                dit[:, bi:bi + 1])
                    nc.sync.dma_start(
                        s2_loc[bi * 128:(bi + 1) * 128,
                               ch * FCH:(ch + 1) * FCH], s2t[:])

            # ---- Phase W2: out = sigmoid(blockdiag(W2,W2)^T @ s2^T + b2) ----
            for p in range(PAIRS):
                s2T = pool_s2t.tile([128, BPC * 128], bf16, tag="s2T")
                nc.sync.dma_start(s2T[:],
                                  s2_loc[:, p * 128:(p + 1) * 128],
                                  transpose=True)
                ps = pool_psum.tile([128, BPC * 128], f32, tag="mm")
                for k0, w in ((0, 512), (512, 512), (1024, 256)):
                    nc.tensor.matmul(ps[:, k0:k0 + w], w2t[:],
                                     s2T[:, k0:k0 + w], start=True, stop=True)
                ot = pool_out.tile([128, BPC * 128], f32, tag="outp")
                nc.scalar.activation(ot[:], ps[:],
                                     mybir.ActivationFunctionType.Sigmoid,
                                     bias=b2t[:])
                nc.sync.dma_start(out_ext[p], ot[:])

    nc.compile()
    return nc


def prepare_inputs(X, edge_index, W1, b1, W2, b2):
    """Host-side graph/layout prep. Returns per-core in_maps."""
    X = np.asarray(X, dtype=np.float32)
    edge_index = np.asarray(edge_index)
    W1 = np.asarray(W1, dtype=np.float32)
    b1 = np.asarray(b1, dtype=np.float32)
    W2 = np.asarray(W2, dtype=np.float32)
    b2 = np.asarray(b2, dtype=np.float32)

    src = edge_index[0].astype(np.int64)
    dst = edge_index[1].astype(np.int64)

    deg = np.bincount(dst, minlength=N).astype(np.float32) + 1.0
    dinv = 1.0 / np.sqrt(deg)
    dinv_pad = np.zeros(NP, np.float32)
    dinv_pad[:N] = dinv

    # M = Adj + I with multiplicity, uint8 counts
    Mfull = np.zeros((NP, NP), np.uint8)
    np.add.at(Mfull, (dst, src), 1)
    Mfull[np.arange(N), np.arange(N)] += 1
    assert Mfull.max() <= 15, "fp8e4 exact-int range exceeded"

    # XT pairs: [12, 128, NP] bf16, dinv-src folded in
    Xs = X * dinv[None, :, None, None]                  # [B, N, T, C]
    XT = np.zeros((S, C, NP), np.float32)
    XT[:, :, :N] = np.transpose(Xs, (0, 2, 3, 1)).reshape(S, C, N)
    XT_pairs = XT.reshape(PAIRS, 128, NP).astype(ml_dtypes.bfloat16)

    def blockdiag(W):
        D = np.zeros((128, 128), np.float32)
        D[:64, :64] = W
        D[64:, 64:] = W
        return D.astype(ml_dtypes.bfloat16)

    W1d = blockdiag(W1)
    W2d = blockdiag(W2)
    B1 = np.tile(b1, (128, FCH // C)).astype(np.float32)
    B2 = np.concatenate([b2, b2])[:, None].astype(np.float32)

    in_maps = []
    for c in range(N_CORES):
        rows = Mfull[c * BPC * 128:(c + 1) * BPC * 128, :]
        Mc = rows.reshape(BPC, 128, NB, 128).transpose(0, 2, 3, 1)
        Mc = np.ascontiguousarray(Mc).astype(ml_dtypes.float8_e4m3)
        DI = dinv_pad[c * BPC * 128:(c + 1) * BPC * 128]
        DI = DI.reshape(BPC, 128).T.astype(np.float32)
        DI = np.ascontiguousarray(DI)
        in_maps.append({"XT": XT_pairs, "M": Mc, "W1d": W1d, "W2d": W2d,
                        "B1": B1, "B2": B2, "DI": DI})
    return in_maps


_NC_CACHE = {}


def kernel(X, edge_index, W1, b1, W2, b2):
    if "nc" not in _NC_CACHE:
        _NC_CACHE["nc"] = build_program(with_collective=True)
    nc = _NC_CACHE["nc"]
    in_maps = prepare_inputs(X, edge_index, W1, b1, W2, b2)

    res = None
    for attempt in range(4):
        try:
            res = run_bass_kernel_spmd(nc, in_maps, list(range(N_CORES)))
            break
        except Exception:
            if attempt == 3:
                raise
            time.sleep(20.0 * (attempt + 1))
    assert res is not None

    # reassemble: per core [12, 128, 1280] -> [24, 64, 1280]
    full = np.zeros((S, C, N), np.float32)
    for c in range(N_CORES):
        o = res.results[c]["OUT"].reshape(S, C, BPC * 128)
        lo = c * BPC * 128
        hi = min(N, (c + 1) * BPC * 128)
        if lo < N:
            full[:, :, lo:hi] = o[:, :, :hi - lo]
    out = full.reshape(B, T, C, N).transpose(0, 3, 1, 2)
    return np.ascontiguousarray(out)


# revision 4
# speedup vs baseline: 1.2284x; 1.2284x over previous
"""GCN block (2-layer) Trainium2 Bass kernel.

Math (per B*T slice, shared graph):
  t2 = relu(A @ (X @ W1) + b1);  out = sigmoid(A @ t2 @ W2 + b2)
  A = D^-1/2 (Adj + I) D^-1/2  (PyG gcn_norm, counts edge multiplicity)

Device mapping:
  A is applied as dense 128x128 blocks of the integer matrix M = Adj + I
  (exact in fp8e4) via PE matmuls accumulating in PSUM; the D^-1/2 factors
  are folded in on the src side (host, into the X cast) and dst side
  (per-partition scale at the PSUM drain).  Layer order is rearranged as
  L1: (X@W1) then A;  L2: A then @W2 — associativity keeps it exact.

Sharding: each of 8 cores owns 10 of the 80 dst-node blocks (128 nodes
each, N padded 10000->10240) for ALL 24 B*T slices; an AllGather of the
relu'd layer-1 activations runs between the layers.

Layout: the 1536 free columns (24 slices x 64 features, slice-major) are
processed in 2 chunks of 768.  W-stages use block_diag(W, W) stationaries
so two slices share full 128-partition matmuls.  W1 runs with X-blocks as
the stationary operand so xw lands directly in node-major SBUF tiles (no
DRAM round trip / DMA transpose).
"""
import time

import numpy as np
import ml_dtypes

import concourse.bacc as bacc
import concourse.mybir as mybir
import concourse.tile as tile
from concourse.bass_utils import run_bass_kernel_spmd

N_CORES = 8
N = 10000
NP = 10240            # padded nodes
NB = NP // 128        # 80 node blocks
BPC = NB // N_CORES   # 10 dst blocks per core
B, T, C = 2, 12, 64
S = B * T             # 24 slices
F = S * C             # 1536 free columns (slice-major: s*64+f)
CH = 2                # free chunks
FCH = F // CH         # 768 cols per chunk = 6 slice pairs
PPC = FCH // 128      # 6 pairs per chunk

f32 = mybir.dt.float32
bf16 = mybir.dt.bfloat16
fp8 = mybir.dt.float8e4


def build_program(with_collective=True):
    nc = bacc.Bacc("TRN2", target_bir_lowering=False, debug=False,
                   num_devices=N_CORES)

    # Xb: per (chunk, src block): stationary [128=(half,cin), PPC, 128 nodes]
    xb_ext = nc.dram_tensor("XB", [CH, NB, 128, PPC * 128], bf16,
                            kind="ExternalInput")
    # M rows: [bi][p_src][bj*128+q_dst], fp8 exact ints
    m_ext = nc.dram_tensor("M", [BPC, 128, NB * 128], fp8, kind="ExternalInput")
    w1_ext = nc.dram_tensor("W1d", [128, 128], bf16, kind="ExternalInput")
    w2_ext = nc.dram_tensor("W2d", [128, 128], bf16, kind="ExternalInput")
    b1_ext = nc.dram_tensor("B1", [128, FCH], f32, kind="ExternalInput")
    b2_ext = nc.dram_tensor("B2", [128, 1], f32, kind="ExternalInput")
    di_ext = nc.dram_tensor("DI", [128, BPC], f32, kind="ExternalInput")
    out_ext = nc.dram_tensor("OUT", [S // 2, 128, BPC * 128], f32,
                             kind="ExternalOutput")

    with tile.TileContext(nc) as tc:
        with (
            tc.tile_pool(name="consts", bufs=1) as consts,
            tc.tile_pool(name="xb", bufs=3) as pool_xb,
            tc.tile_pool(name="xw", bufs=NB + 2) as pool_xw,
            tc.tile_pool(name="m", bufs=2) as pool_m,
            tc.tile_pool(name="u", bufs=2) as pool_u,
            tc.tile_pool(name="t2", bufs=3) as pool_t2,
            tc.tile_pool(name="s2T", bufs=2) as pool_s2t,
            tc.tile_pool(name="outp", bufs=2) as pool_out,
            tc.tile_pool(name="pA", bufs=2, space="PSUM") as pool_pa,
            tc.tile_pool(name="pW2", bufs=1, space="PSUM") as pool_pw2,
            tc.tile_pool(name="dram", bufs=1, space="DRAM") as dram,
        ):
            # constants
            w1t = consts.tile([128, 128], bf16, tag="w1")
            nc.sync.dma_start(w1t[:], w1_ext[:])
            w2t = consts.tile([128, 128], bf16, tag="w2")
            nc.sync.dma_start(w2t[:], w2_ext[:])
            b1t = consts.tile([128, FCH], f32, tag="b1")
            nc.sync.dma_start(b1t[:], b1_ext[:])
            b2t = consts.tile([128, 1], f32, tag="b2")
            nc.sync.dma_start(b2t[:], b2_ext[:])
            dit = consts.tile([128, BPC], f32, tag="di")
            nc.sync.dma_start(dit[:], di_ext[:])

            # DRAM intermediates
            t2_loc = dram.tile([BPC * 128, F], bf16, tag="t2loc")
            if with_collective:
                t2_full = dram.tile([NP, F], bf16, tag="t2full",
                                    addr_space="Shared")
            else:
                t2_full = dram.tile([NP, F], bf16, tag="t2full")
            s2_loc = dram.tile([BPC * 128, F], bf16, tag="s2loc")

            # ---- Layer 1: W1 then A1, chunk by chunk ----
            for ch in range(CH):
                # W1: xwt[bj] = (Xb_bj)^T @ blockdiag(W1,W1), node-major
                xwt = []
                for bj in range(NB):
                    xb = pool_xb.tile([128, PPC, 128], bf16, tag="xb")
                    nc.sync.dma_start(xb[:], xb_ext[ch, bj].rearrange(
                        "p (a d) -> p a d", a=PPC))
                    ps = pool_pa.tile([128, FCH], f32, tag="mm")
                    for pl in range(PPC):
                        nc.tensor.matmul(ps[:, pl * 128:(pl + 1) * 128],
                                         xb[:, pl, :], w1t[:],
                                         start=True, stop=True)
                    t = pool_xw.tile([128, FCH], bf16, tag="xw")
                    if bj % 2 == 0:
                        nc.vector.tensor_scalar_mul(t[:], ps[:], 1.0)
                    else:
                        nc.scalar.activation(t[:], ps[:],
                                             mybir.ActivationFunctionType.Copy)
                    xwt.append(t)
                # A1: psum_bi = sum_bj MT[bi,bj] @ xwt[bj]
                for bi in range(BPC):
                    mrow = pool_m.tile([128, NB * 128], fp8, tag="m")
                    nc.scalar.dma_start(mrow[:], m_ext[bi])
                    ps = pool_pa.tile([128, FCH], f32, tag="mm")
                    for c0, w in ((0, 512), (512, FCH - 512)):
                        for bj in range(NB):
                            nc.tensor.matmul(
                                ps[:, c0:c0 + w],
                                mrow[:, bj * 128:(bj + 1) * 128],
                                xwt[bj][:, c0:c0 + w],
                                start=(bj == 0), stop=(bj == NB - 1))
                    u = pool_u.tile([128, FCH], f32, tag="u")
                    nc.vector.scalar_tensor_tensor(
                        u[:], ps[:], dit[:, bi:bi + 1], b1t[:],
                        mybir.AluOpType.mult, mybir.AluOpType.add)
                    t2t = pool_t2.tile([128, FCH], bf16, tag="t2")
                    nc.scalar.activation(t2t[:], u[:],
                                         mybir.ActivationFunctionType.Relu,
                                         scale=dit[:, bi:bi + 1])
                    nc.gpsimd.dma_start(
                        t2_loc[bi * 128:(bi + 1) * 128,
                               ch * FCH:(ch + 1) * FCH], t2t[:])

            # ---- AllGather t2 across the 8 cores ----
            if with_collective:
                nc.gpsimd.collective_compute(
                    "AllGather", mybir.AluOpType.bypass,
                    replica_groups=[list(range(N_CORES))],
                    ins=[t2_loc[:]], outs=[t2_full[:]])
            else:
                # timing stand-in: emulate receive traffic
                for r in range(N_CORES):
                    nc.gpsimd.dma_start(
                        t2_full[r * BPC * 128:(r + 1) * BPC * 128, :],
                        t2_loc[:])

            # ---- Layer 2 A-stage: s2 = dinv*(M @ t2) ----
            for ch in range(CH):
                t2c = []
                for bj in range(NB):
                    t = pool_xw.tile([128, FCH], bf16, tag="xw")
                    nc.sync.dma_start(
                        t[:], t2_full[bj * 128:(bj + 1) * 128,
                                      ch * FCH:(ch + 1) * FCH])
                    t2c.append(t)
                for bi in range(BPC):
                    mrow = pool_m.tile([128, NB * 128], fp8, tag="m")
                    nc.scalar.dma_start(mrow[:], m_ext[bi])
                    ps = pool_pa.tile([128, FCH], f32, tag="mm")
                    for c0, w in ((0, 512), (512, FCH - 512)):
                        for bj in range(NB):
                            nc.tensor.matmul(
                                ps[:, c0:c0 + w],
                                mrow[:, bj * 128:(bj + 1) * 128],
                                t2c[bj][:, c0:c0 + w],
                                start=(bj == 0), stop=(bj == NB - 1))
                    s2t = pool_t2.tile([128, FCH], bf16, tag="t2")
                    nc.vector.tensor_scalar_mul(s2t[:], ps[:],
                                                dit[:, bi:bi + 1])
                    nc.gpsimd.dma_start(
                        s2_loc[bi * 128:(bi + 1) * 128,
                               ch * FCH:(ch + 1) * FCH], s2t[:])

            # ---- W2: out = sigmoid(blockdiag(W2,W2)^T @ s2^T + b2) ----
            for p in range(S // 2):
                s2T = pool_s2t.tile([128, BPC * 128], bf16, tag="s2T")
                nc.sync.dma_start(s2T[:],
                                  s2_loc[:, p * 128:(p + 1) * 128],
                                  transpose=True)
                ps = pool_pw2.tile([128, BPC * 128], f32, tag="w2")
                for k0, w in ((0, 512), (512, 512), (1024, 256)):
                    nc.tensor.matmul(ps[:, k0:k0 + w], w2t[:],
                                     s2T[:, k0:k0 + w], start=True, stop=True)
                ot = pool_out.tile([128, BPC * 128], f32, tag="outp")
                nc.scalar.activation(ot[:], ps[:],
                                     mybir.ActivationFunctionType.Sigmoid,
                                     bias=b2t[:])
                nc.gpsimd.dma_start(out_ext[p], ot[:])

    nc.compile()
    return nc


def prepare_inputs(X, edge_index, W1, b1, W2, b2):
    """Host-side graph/layout prep. Returns per-core in_maps."""
    X = np.asarray(X, dtype=np.float32)
    edge_index = np.asarray(edge_index)
    W1 = np.asarray(W1, dtype=np.float32)
    b1 = np.asarray(b1, dtype=np.float32)
    W2 = np.asarray(W2, dtype=np.float32)
    b2 = np.asarray(b2, dtype=np.float32)

    src = edge_index[0].astype(np.int64)
    dst = edge_index[1].astype(np.int64)

    deg = np.bincount(dst, minlength=N).astype(np.float32) + 1.0
    dinv = 1.0 / np.sqrt(deg)
    dinv_pad = np.zeros(NP, np.float32)
    dinv_pad[:N] = dinv

    # M = Adj + I with multiplicity, uint8 counts
    Mfull = np.zeros((NP, NP), np.uint8)
    np.add.at(Mfull, (dst, src), 1)
    Mfull[np.arange(N), np.arange(N)] += 1
    assert Mfull.max() <= 15, "fp8e4 exact-int range exceeded"

    # XB: [CH, NB, 128=(h,cin), PPC*128] with dinv-src folded in
    Xs = X * dinv[None, :, None, None]                  # [B, N, T, C]
    XT = np.zeros((S, C, NP), np.float32)
    XT[:, :, :N] = np.transpose(Xs, (0, 2, 3, 1)).reshape(S, C, N)
    # s = FCH//64*ch... slice order within chunk: s = 12*ch + 2*pl + h
    x6 = XT.reshape(CH, PPC, 2, C, NB, 128)
    XB = np.ascontiguousarray(np.transpose(x6, (0, 4, 2, 3, 1, 5)))
    XB = XB.reshape(CH, NB, 128, PPC * 128).astype(ml_dtypes.bfloat16)

    def blockdiag(W):
        D = np.zeros((128, 128), np.float32)
        D[:64, :64] = W
        D[64:, 64:] = W
        return D.astype(ml_dtypes.bfloat16)

    W1d = blockdiag(W1)
    W2d = blockdiag(W2)
    B1 = np.tile(b1, (128, FCH // C)).astype(np.float32)
    B2 = np.concatenate([b2, b2])[:, None].astype(np.float32)

    in_maps = []
    for c in range(N_CORES):
        rows = Mfull[c * BPC * 128:(c + 1) * BPC * 128, :]
        Mc = rows.reshape(BPC, 128, NB, 128).transpose(0, 3, 2, 1)
        Mc = np.ascontiguousarray(Mc).reshape(BPC, 128, NB * 128)
        Mc = Mc.astype(ml_dtypes.float8_e4m3)
        DI = dinv_pad[c * BPC * 128:(c + 1) * BPC * 128]
        DI = DI.reshape(BPC, 128).T.astype(np.float32)
        DI = np.ascontiguousarray(DI)
        in_maps.append({"XB": XB, "M": Mc, "W1d": W1d, "W2d": W2d,
                        "B1": B1, "B2": B2, "DI": DI})
    return in_maps


_NC_CACHE = {}


def kernel(X, edge_index, W1, b1, W2, b2):
    if "nc" not in _NC_CACHE:
        _NC_CACHE["nc"] = build_program(with_collective=True)
    nc = _NC_CACHE["nc"]
    in_maps = prepare_inputs(X, edge_index, W1, b1, W2, b2)

    res = None
    for attempt in range(5):
        try:
            res = run_bass_kernel_spmd(nc, in_maps, list(range(N_CORES)))
            break
        except Exception:
            if attempt == 4:
                raise
            time.sleep(60.0 * (attempt + 1))
    assert res is not None

    # reassemble: per core [12, 128, 1280] -> [24, 64, 1280]
    full = np.zeros((S, C, N), np.float32)
    for c in range(N_CORES):
        o = res.results[c]["OUT"].reshape(S, C, BPC * 128)
        lo = c * BPC * 128
        hi = min(N, (c + 1) * BPC * 128)
        if lo < N:
            full[:, :, lo:hi] = o[:, :, :hi - lo]
    out = full.reshape(B, T, C, N).transpose(0, 3, 1, 2)
    return np.ascontiguousarray(out)


# revision 6
# speedup vs baseline: 2.7353x; 2.2268x over previous
"""GCN block (2-layer) Trainium2 Bass kernel.

Math (per B*T slice, shared graph):
  t2 = relu(A @ (X @ W1) + b1);  out = sigmoid(A @ t2 @ W2 + b2)
  A = D^-1/2 (Adj + I) D^-1/2  (PyG gcn_norm, counts edge multiplicity)

Device mapping:
  A is applied as dense 128x128 blocks of the integer matrix M = Adj + I
  (exact in fp8e4) via PE matmuls accumulating in PSUM; the D^-1/2 factors
  are folded in on the src side (host, into the X cast) and dst side
  (per-partition scale at the PSUM drain).  Layer order is rearranged as
  L1: (X@W1) then A;  L2: A then @W2 — associativity keeps it exact.
  The A-stage matmuls run in fp8 DoubleRow mode (K=256: two 128-node src
  blocks per matmul, weights exact small ints in fp8e4).

Sharding: each of 8 cores owns 10 of the 80 dst-node blocks (128 nodes
each, N padded 10000->10240) for ALL 24 B*T slices; an AllGather of the
relu'd layer-1 activations runs between the layers.

Layout: the 1536 free columns (24 slices x 64 features, slice-major) are
processed in 2 chunks of 768.  W-stages use block_diag(W, W) stationaries
so two slices share full 128-partition matmuls.  W1 runs with X-blocks as
the stationary operand so xw lands directly in node-major SBUF tiles (no
DRAM round trip / DMA transpose).
"""
import time

import numpy as np
import ml_dtypes

import concourse.bacc as bacc
import concourse.mybir as mybir
import concourse.tile as tile
from concourse.bass_utils import run_bass_kernel_spmd

N_CORES = 8
N = 10000
NP = 10240            # padded nodes
NB = NP // 128        # 80 node blocks
NB2 = NB // 2         # 40 src-block pairs (DoubleRow K=256)
BPC = NB // N_CORES   # 10 dst blocks per core
B, T, C = 2, 12, 64
S = B * T             # 24 slices
F = S * C             # 1536 free columns (slice-major: s*64+f)
CH = 2                # free chunks
FCH = F // CH         # 768 cols per chunk = 6 slice pairs
PPC = FCH // 128      # 6 pairs per chunk

f32 = mybir.dt.float32
bf16 = mybir.dt.bfloat16
fp8 = mybir.dt.float8e4
DR = mybir.MatmulPerfMode.DoubleRow


def build_program(with_collective=True, use_dr=True, nc_hook=None):
    """use_dr: fp8 DoubleRow A-stage (moving operand quantized to fp8e4).
    use_dr=False falls back to bf16 moving operands (more exact, 4x PE)."""
    mdt = fp8 if use_dr else bf16
    nc = bacc.Bacc("TRN2", target_bir_lowering=False, debug=False,
                   num_devices=N_CORES)
    if nc_hook is not None:
        nc_hook(nc)

    # Xb: per (chunk, src block): stationary [128=(half,cin), PPC, 128 nodes]
    xb_ext = nc.dram_tensor("XB", [CH, NB, 128, PPC * 128], bf16,
                            kind="ExternalInput")
    # M rows: [bi][p_src][bj*128+q_dst], fp8 exact ints
    m_ext = nc.dram_tensor("M", [BPC, 128, NB * 128], fp8, kind="ExternalInput")
    w1_ext = nc.dram_tensor("W1d", [128, 128], bf16, kind="ExternalInput")
    w2_ext = nc.dram_tensor("W2d", [128, 128], bf16, kind="ExternalInput")
    b1_ext = nc.dram_tensor("B1", [128, FCH], f32, kind="ExternalInput")
    b2_ext = nc.dram_tensor("B2", [128, 1], f32, kind="ExternalInput")
    di_ext = nc.dram_tensor("DI", [128, BPC], f32, kind="ExternalInput")
    out_ext = nc.dram_tensor("OUT", [S // 2, 128, BPC * 128], f32,
                             kind="ExternalOutput")

    with tile.TileContext(nc) as tc:
        with (
            tc.tile_pool(name="consts", bufs=1) as consts,
            tc.tile_pool(name="xb", bufs=4) as pool_xb,
            tc.tile_pool(name="xw", bufs=NB2 + 2) as pool_xw,
            tc.tile_pool(name="m", bufs=2) as pool_m,
            tc.tile_pool(name="u", bufs=2) as pool_u,
            tc.tile_pool(name="t2", bufs=3) as pool_t2,
            tc.tile_pool(name="s2T", bufs=2) as pool_s2t,
            tc.tile_pool(name="outp", bufs=2) as pool_out,
            tc.tile_pool(name="pA", bufs=2, space="PSUM") as pool_pa,
            tc.tile_pool(name="pW2", bufs=1, space="PSUM") as pool_pw2,
            tc.tile_pool(name="dram", bufs=1, space="DRAM") as dram,
        ):
            # constants
            w1t = consts.tile([128, 128], bf16, tag="w1")
            nc.sync.dma_start(w1t[:], w1_ext[:])
            w2t = consts.tile([128, 128], bf16, tag="w2")
            nc.sync.dma_start(w2t[:], w2_ext[:])
            b1t = consts.tile([128, FCH], f32, tag="b1")
            nc.sync.dma_start(b1t[:], b1_ext[:])
            b2t = consts.tile([128, 1], f32, tag="b2")
            nc.sync.dma_start(b2t[:], b2_ext[:])
            dit = consts.tile([128, BPC], f32, tag="di")
            nc.sync.dma_start(dit[:], di_ext[:])

            # DRAM intermediates
            t2_loc = dram.tile([BPC * 128, F], mdt, tag="t2loc")
            if with_collective:
                t2_full = dram.tile([NP, F], mdt, tag="t2full",
                                    addr_space="Shared")
            else:
                t2_full = dram.tile([NP, F], mdt, tag="t2full")
            s2_loc = dram.tile([BPC * 128, F], bf16, tag="s2loc")

            def a_stage_mms(ps, mrow, pair_tiles):
                """psum += M @ x over all src blocks (DoubleRow or plain)."""
                for c0, w in ((0, 512), (512, FCH - 512)):
                    if use_dr:
                        for j2 in range(NB2):
                            nc.tensor.matmul(
                                ps[:, c0:c0 + w], mrow[:, j2],
                                pair_tiles[j2][:, :, c0:c0 + w],
                                start=(j2 == 0), stop=(j2 == NB2 - 1),
                                perf_mode=DR)
                    else:
                        for bj in range(NB):
                            nc.tensor.matmul(
                                ps[:, c0:c0 + w],
                                mrow[:, bj // 2, bj % 2, :],
                                pair_tiles[bj // 2][:, bj % 2, c0:c0 + w],
                                start=(bj == 0), stop=(bj == NB - 1))

            # ---- Layer 1: W1 then A1, chunk by chunk ----
            for ch in range(CH):
                # W1: xw pair tiles [128, 2, FCH], node-major, dtype mdt
                xwt = []
                for j2 in range(NB2):
                    pt = pool_xw.tile([128, 2, FCH], mdt, tag="xw")
                    for ko in range(2):
                        bj = 2 * j2 + ko
                        xb = pool_xb.tile([128, PPC, 128], bf16, tag="xb")
                        nc.sync.dma_start(xb[:], xb_ext[ch, bj].rearrange(
                            "p (a d) -> p a d", a=PPC))
                        ps = pool_pa.tile([128, FCH], f32, tag="mm")
                        for pl in range(PPC):
                            nc.tensor.matmul(ps[:, pl * 128:(pl + 1) * 128],
                                             xb[:, pl, :], w1t[:],
                                             start=True, stop=True)
                        if bj % 2 == 0:
                            nc.vector.tensor_scalar_mul(pt[:, ko, :], ps[:], 1.0)
                        else:
                            nc.scalar.activation(
                                pt[:, ko, :], ps[:],
                                mybir.ActivationFunctionType.Copy)
                    xwt.append(pt)
                # A1: psum_bi = sum_bj MT[bi,bj] @ xw[bj]
                for bi in range(BPC):
                    mrow = pool_m.tile([128, NB2, 2, 128], fp8, tag="m")
                    nc.scalar.dma_start(
                        mrow[:].rearrange("p a b q -> p (a b q)"), m_ext[bi])
                    ps = pool_pa.tile([128, FCH], f32, tag="mm")
                    a_stage_mms(ps, mrow, xwt)
                    u = pool_u.tile([128, FCH], f32, tag="u")
                    nc.vector.scalar_tensor_tensor(
                        u[:], ps[:], dit[:, bi:bi + 1], b1t[:],
                        mybir.AluOpType.mult, mybir.AluOpType.add)
                    t2t = pool_t2.tile([128, FCH], mdt, tag="t2")
                    nc.scalar.activation(t2t[:], u[:],
                                         mybir.ActivationFunctionType.Relu,
                                         scale=dit[:, bi:bi + 1])
                    nc.gpsimd.dma_start(
                        t2_loc[bi * 128:(bi + 1) * 128,
                               ch * FCH:(ch + 1) * FCH], t2t[:])

            # ---- AllGather t2 across the 8 cores ----
            if with_collective:
                nc.gpsimd.collective_compute(
                    "AllGather", mybir.AluOpType.bypass,
                    replica_groups=[list(range(N_CORES))],
                    ins=[t2_loc[:]], outs=[t2_full[:]])
            else:
                # timing stand-in: emulate receive traffic
                for r in range(N_CORES):
                    nc.gpsimd.dma_start(
                        t2_full[r * BPC * 128:(r + 1) * BPC * 128, :],
                        t2_loc[:])

            # ---- Layer 2 A-stage: s2 = dinv*(M @ t2) ----
            for ch in range(CH):
                t2c = []
                for j2 in range(NB2):
                    pt = pool_xw.tile([128, 2, FCH], mdt, tag="xw")
                    for ko in range(2):
                        bj = 2 * j2 + ko
                        nc.sync.dma_start(
                            pt[:, ko, :],
                            t2_full[bj * 128:(bj + 1) * 128,
                                    ch * FCH:(ch + 1) * FCH])
                    t2c.append(pt)
                for bi in range(BPC):
                    mrow = pool_m.tile([128, NB2, 2, 128], fp8, tag="m")
                    nc.scalar.dma_start(
                        mrow[:].rearrange("p a b q -> p (a b q)"), m_ext[bi])
                    ps = pool_pa.tile([128, FCH], f32, tag="mm")
                    a_stage_mms(ps, mrow, t2c)
                    s2t = pool_t2.tile([128, FCH], bf16, tag="t2b")
                    nc.vector.tensor_scalar_mul(s2t[:], ps[:],
                                                dit[:, bi:bi + 1])
                    nc.gpsimd.dma_start(
                        s2_loc[bi * 128:(bi + 1) * 128,
                               ch * FCH:(ch + 1) * FCH], s2t[:])

            # ---- W2: out = sigmoid(blockdiag(W2,W2)^T @ s2^T + b2) ----
            for p in range(S // 2):
                s2T = pool_s2t.tile([128, BPC * 128], bf16, tag="s2T")
                nc.sync.dma_start(s2T[:],
                                  s2_loc[:, p * 128:(p + 1) * 128],
                                  transpose=True)
                ps = pool_pw2.tile([128, BPC * 128], f32, tag="w2")
                for k0, w in ((0, 512), (512, 512), (1024, 256)):
                    nc.tensor.matmul(ps[:, k0:k0 + w], w2t[:],
                                     s2T[:, k0:k0 + w], start=True, stop=True)
                ot = pool_out.tile([128, BPC * 128], f32, tag="outp")
                nc.scalar.activation(ot[:], ps[:],
                                     mybir.ActivationFunctionType.Sigmoid,
                                     bias=b2t[:])
                nc.gpsimd.dma_start(out_ext[p], ot[:])

    nc.compile()
    return nc


def prepare_inputs(X, edge_index, W1, b1, W2, b2):
    """Host-side graph/layout prep. Returns per-core in_maps."""
    X = np.asarray(X, dtype=np.float32)
    edge_index = np.asarray(edge_index)
    W1 = np.asarray(W1, dtype=np.float32)
    b1 = np.asarray(b1, dtype=np.float32)
    W2 = np.asarray(W2, dtype=np.float32)
    b2 = np.asarray(b2, dtype=np.float32)

    src = edge_index[0].astype(np.int64)
    dst = edge_index[1].astype(np.int64)

    deg = np.bincount(dst, minlength=N).astype(np.float32) + 1.0
    dinv = 1.0 / np.sqrt(deg)
    dinv_pad = np.zeros(NP, np.float32)
    dinv_pad[:N] = dinv

    # M = Adj + I with multiplicity, uint8 counts
    Mfull = np.zeros((NP, NP), np.uint8)
    np.add.at(Mfull, (dst, src), 1)
    Mfull[np.arange(N), np.arange(N)] += 1
    assert Mfull.max() <= 15, "fp8e4 exact-int range exceeded"

    # XB: [CH, NB, 128=(h,cin), PPC*128] with dinv-src folded in
    Xs = X * dinv[None, :, None, None]                  # [B, N, T, C]
    XT = np.zeros((S, C, NP), np.float32)
    XT[:, :, :N] = np.transpose(Xs, (0, 2, 3, 1)).reshape(S, C, N)
    # slice order within chunk: s = 12*ch + 2*pl + h
    x6 = XT.reshape(CH, PPC, 2, C, NB, 128)
    XB = np.ascontiguousarray(np.transpose(x6, (0, 4, 2, 3, 1, 5)))
    XB = XB.reshape(CH, NB, 128, PPC * 128).astype(ml_dtypes.bfloat16)

    def blockdiag(W):
        D = np.zeros((128, 128), np.float32)
        D[:64, :64] = W
        D[64:, 64:] = W
        return D.astype(ml_dtypes.bfloat16)

    W1d = blockdiag(W1)
    W2d = blockdiag(W2)
    B1 = np.tile(b1, (128, FCH // C)).astype(np.float32)
    B2 = np.concatenate([b2, b2])[:, None].astype(np.float32)

    in_maps = []
    for c in range(N_CORES):
        rows = Mfull[c * BPC * 128:(c + 1) * BPC * 128, :]
        Mc = rows.reshape(BPC, 128, NB, 128).transpose(0, 3, 2, 1)
        Mc = np.ascontiguousarray(Mc).reshape(BPC, 128, NB * 128)
        Mc = Mc.astype(ml_dtypes.float8_e4m3)
        DI = dinv_pad[c * BPC * 128:(c + 1) * BPC * 128]
        DI = DI.reshape(BPC, 128).T.astype(np.float32)
        DI = np.ascontiguousarray(DI)
        in_maps.append({"XB": XB, "M": Mc, "W1d": W1d, "W2d": W2d,
                        "B1": B1, "B2": B2, "DI": DI})
    return in_maps


_NC_CACHE = {}


def kernel(X, edge_index, W1, b1, W2, b2):
    if "nc" not in _NC_CACHE:
        _NC_CACHE["nc"] = build_program(with_collective=True)
    nc = _NC_CACHE["nc"]
    in_maps = prepare_inputs(X, edge_index, W1, b1, W2, b2)

    res = None
    for attempt in range(5):
        try:
            res = run_bass_kernel_spmd(nc, in_maps, list(range(N_CORES)))
            break
        except Exception:
            if attempt == 4:
                raise
            time.sleep(60.0 * (attempt + 1))
    assert res is not None

    # reassemble: per core [12, 128, 1280] -> [24, 64, 1280]
    full = np.zeros((S, C, N), np.float32)
    for c in range(N_CORES):
        o = res.results[c]["OUT"].reshape(S, C, BPC * 128)
        lo = c * BPC * 128
        hi = min(N, (c + 1) * BPC * 128)
        if lo < N:
            full[:, :, lo:hi] = o[:, :, :hi - lo]
    out = full.reshape(B, T, C, N).transpose(0, 3, 1, 2)
    return np.ascontiguousarray(out)


# revision 13
# speedup vs baseline: 3.0578x; 1.1179x over previous
"""GCN block (2-layer) Trainium2 Bass kernel.

Math (per B*T slice, shared graph):
  t2 = relu(A @ (X @ W1) + b1);  out = sigmoid(A @ t2 @ W2 + b2)
  A = D^-1/2 (Adj + I) D^-1/2  (PyG gcn_norm, counts edge multiplicity)

Device mapping:
  A is applied as dense 128x128 blocks of the integer matrix M = Adj + I
  (exact in fp8e4) via PE matmuls accumulating in PSUM; the D^-1/2 factors
  are folded in on the src side (host, into the X cast) and dst side
  (per-partition scale at the PSUM drain).  Layer order is rearranged as
  L1: (X@W1) then A;  L2: A then @W2 — associativity keeps it exact.
  The A-stage matmuls run in fp8 DoubleRow mode (K=256: two 128-node src
  blocks per matmul, weights exact small ints in fp8e4).

Sharding: each of 8 cores owns 10 of the 80 dst-node blocks (128 nodes
each, N padded 10000->10240) for ALL 24 B*T slices; an AllGather of the
relu'd layer-1 activations runs between the layers.

Layout: all 1536 free columns (24 slices x 64 features, slice-major
s*64+f with s = 2*pl+h) stay resident; the moving operand lives in 20
"quad" SBUF tiles [128, 4 src blocks, 1536] fp8.  W-stages use
block_diag(W, W) stationaries so two slices share full 128-partition
matmuls; W1 runs with X-blocks as the stationary so xw lands directly in
node-major SBUF (no DRAM round trip / transpose).
"""
import time

import numpy as np
import ml_dtypes

import concourse.bacc as bacc
import concourse.mybir as mybir
import concourse.tile as tile
from concourse.bass_utils import run_bass_kernel_spmd

N_CORES = 8
N = 10000
NP = 10240            # padded nodes
NB = NP // 128        # 80 node blocks
NB2 = NB // 2         # 40 src-block pairs (DoubleRow K=256)
NQ = NB // 4          # 20 quad tiles
BPC = NB // N_CORES   # 10 dst blocks per core
B, T, C = 2, 12, 64
S = B * T             # 24 slices
F = S * C             # 1536 free columns
PAIRS = S // 2        # 12 slice pairs (pl)
CHAINS = ((0, 512), (512, 512), (1024, 512))

f32 = mybir.dt.float32
bf16 = mybir.dt.bfloat16
fp8 = mybir.dt.float8e4
DR = mybir.MatmulPerfMode.DoubleRow


def build_program(with_collective=True, nc_hook=None):
    nc = bacc.Bacc("TRN2", target_bir_lowering=False, debug=False,
                   num_devices=N_CORES)
    if nc_hook is not None:
        nc_hook(nc)

    # X blocks: [bj][128=(h,cin)][pl*128+node], stationary operands for W1
    xb_ext = nc.dram_tensor("XB", [NB, 128, PAIRS * 128], bf16,
                            kind="ExternalInput")
    # M rows: [bi][p_src][bj*128+q_dst], fp8 exact ints
    m_ext = nc.dram_tensor("M", [BPC, 128, NB * 128], fp8, kind="ExternalInput")
    w1_ext = nc.dram_tensor("W1d", [128, 128], bf16, kind="ExternalInput")
    w2_ext = nc.dram_tensor("W2d", [128, 128], bf16, kind="ExternalInput")
    b1_ext = nc.dram_tensor("B1", [128, F], f32, kind="ExternalInput")
    b2_ext = nc.dram_tensor("B2", [128, 1], f32, kind="ExternalInput")
    di_ext = nc.dram_tensor("DI", [128, BPC], f32, kind="ExternalInput")
    out_ext = nc.dram_tensor("OUT", [PAIRS, 128, BPC * 128], f32,
                             kind="ExternalOutput")

    with tile.TileContext(nc) as tc:
        with (
            tc.tile_pool(name="consts", bufs=1) as consts,
            tc.tile_pool(name="xb", bufs=2) as pool_xb,
            tc.tile_pool(name="xw", bufs=NQ + 1) as pool_xw,
            tc.tile_pool(name="m", bufs=2) as pool_m,
            tc.tile_pool(name="u", bufs=3) as pool_u,
            tc.tile_pool(name="t2", bufs=2) as pool_t2,
            tc.tile_pool(name="s2b", bufs=2) as pool_s2b,
            tc.tile_pool(name="s2T", bufs=2) as pool_s2t,
            tc.tile_pool(name="outp", bufs=1) as pool_out,
            tc.tile_pool(name="pw", bufs=2, space="PSUM") as pool_pw,
            tc.tile_pool(name="pa", bufs=2, space="PSUM") as pool_pa,
            tc.tile_pool(name="dram", bufs=1, space="DRAM") as dram,
        ):
            # constants
            w1t = consts.tile([128, 128], bf16, tag="w1")
            nc.sync.dma_start(w1t[:], w1_ext[:])
            w2t = consts.tile([128, 128], bf16, tag="w2")
            nc.sync.dma_start(w2t[:], w2_ext[:])
            b1t = consts.tile([128, F], f32, tag="b1")
            nc.sync.dma_start(b1t[:], b1_ext[:])
            b2t = consts.tile([128, 1], f32, tag="b2")
            nc.sync.dma_start(b2t[:], b2_ext[:])
            dit = consts.tile([128, BPC], f32, tag="di")
            nc.sync.dma_start(dit[:], di_ext[:])

            # DRAM intermediates
            t2_loc = dram.tile([BPC * 128, F], fp8, tag="t2loc")
            if with_collective:
                t2_full = dram.tile([NP, F], fp8, tag="t2full",
                                    addr_space="Shared")
            else:
                t2_full = dram.tile([NP, F], fp8, tag="t2full")
            s2_loc = dram.tile([BPC * 128, F], bf16, tag="s2loc")

            def a_stage_mms(ps_list, mrow, quads):
                """For each 512-col chain: psum += M @ x, fp8 DoubleRow."""
                for ps, (c0, w) in zip(ps_list, CHAINS):
                    for j2 in range(NB2):
                        nc.tensor.matmul(
                            ps[:], mrow[:, j2],
                            quads[j2 // 2][:, 2 * (j2 % 2):2 * (j2 % 2) + 2,
                                           c0:c0 + w],
                            start=(j2 == 0), stop=(j2 == NB2 - 1),
                            perf_mode=DR)

            # ---- Layer 1 W1: xw quads, node-major, fp8 ----
            xwt = []
            for q in range(NQ):
                quad = pool_xw.tile([128, 4, F], fp8, tag="xw")
                for xp in range(2):
                    xb = pool_xb.tile([128, 2, PAIRS * 128], bf16, tag="xb")
                    nc.sync.dma_start(
                        xb[:],
                        xb_ext[4 * q + 2 * xp:4 * q + 2 * xp + 2]
                        .rearrange("a p d -> p a d"))
                    for k2 in range(2):
                        sub = 2 * xp + k2
                        ps = pool_pw.tile([128, F], f32, tag="pw")
                        for pl in range(PAIRS):
                            nc.tensor.matmul(
                                ps[:, pl * 128:(pl + 1) * 128],
                                xb[:, k2, pl * 128:(pl + 1) * 128], w1t[:],
                                start=True, stop=True)
                        if sub % 2 == 0:
                            nc.vector.tensor_scalar_mul(quad[:, sub, :],
                                                        ps[:], 1.0)
                        else:
                            nc.scalar.activation(
                                quad[:, sub, :], ps[:],
                                mybir.ActivationFunctionType.Copy)
                xwt.append(quad)

            # ---- Layer 1 A: t2 = dinv*relu(dinv*(M @ xw) + b1) ----
            for bi in range(BPC):
                mrow = pool_m.tile([128, NB2, 2, 128], fp8, tag="m")
                nc.scalar.dma_start(
                    mrow[:].rearrange("p a b q -> p (a b q)"), m_ext[bi])
                ps_list = [pool_pa.tile([128, w], f32, tag="pa",
                                        name=f"pa{bi}_{k}")
                           for k, (_, w) in enumerate(CHAINS)]
                a_stage_mms(ps_list, mrow, xwt)
                t2t = pool_t2.tile([128, F], fp8, tag="t2")
                for ps, (c0, w) in zip(ps_list, CHAINS):
                    u = pool_u.tile([128, 512], f32, tag="u")
                    nc.vector.scalar_tensor_tensor(
                        u[:, :w], ps[:], dit[:, bi:bi + 1],
                        b1t[:, c0:c0 + w],
                        mybir.AluOpType.mult, mybir.AluOpType.add)
                    nc.scalar.activation(t2t[:, c0:c0 + w], u[:, :w],
                                         mybir.ActivationFunctionType.Relu,
                                         scale=dit[:, bi:bi + 1])
                nc.gpsimd.dma_start(
                    t2_loc[bi * 128:(bi + 1) * 128, :], t2t[:])

            # ---- AllGather t2 across the 8 cores ----
            if with_collective:
                nc.gpsimd.collective_compute(
                    "AllGather", mybir.AluOpType.bypass,
                    replica_groups=[list(range(N_CORES))],
                    ins=[t2_loc[:]], outs=[t2_full[:]])
            else:
                # timing stand-in: emulate receive traffic
                for r in range(N_CORES):
                    nc.gpsimd.dma_start(
                        t2_full[r * BPC * 128:(r + 1) * BPC * 128, :],
                        t2_loc[:])

            # ---- Layer 2 A: s2 = dinv*(M @ t2) ----
            t2c = []
            for q in range(NQ):
                quad = pool_xw.tile([128, 4, F], fp8, tag="xw")
                nc.sync.dma_start(
                    quad[:],
                    t2_full[4 * q * 128:(4 * q + 4) * 128, :]
                    .rearrange("(a p) f -> p a f", p=128))
                t2c.append(quad)
            for bi in range(BPC):
                mrow = pool_m.tile([128, NB2, 2, 128], fp8, tag="m")
                nc.scalar.dma_start(
                    mrow[:].rearrange("p a b q -> p (a b q)"), m_ext[bi])
                ps_list = [pool_pa.tile([128, w], f32, tag="pa",
                                        name=f"pb{bi}_{k}")
                           for k, (_, w) in enumerate(CHAINS)]
                a_stage_mms(ps_list, mrow, t2c)
                s2t = pool_s2b.tile([128, F], bf16, tag="s2b")
                for ps, (c0, w) in zip(ps_list, CHAINS):
                    nc.vector.tensor_scalar_mul(s2t[:, c0:c0 + w], ps[:],
                                                dit[:, bi:bi + 1])
                nc.gpsimd.dma_start(
                    s2_loc[bi * 128:(bi + 1) * 128, :], s2t[:])

            # ---- W2: out = sigmoid(blockdiag(W2,W2)^T @ s2^T + b2) ----
            for p in range(PAIRS):
                s2T = pool_s2t.tile([128, BPC * 128], bf16, tag="s2T")
                nc.sync.dma_start(s2T[:],
                                  s2_loc[:, p * 128:(p + 1) * 128],
                                  transpose=True)
                ps = pool_pw.tile([128, F], f32, tag="pw")
                for k0, w in ((0, 512), (512, 512), (1024, 256)):
                    nc.tensor.matmul(ps[:, k0:k0 + w], w2t[:],
                                     s2T[:, k0:k0 + w], start=True, stop=True)
                ot = pool_out.tile([128, BPC * 128], f32, tag="outp")
                nc.scalar.activation(ot[:], ps[:, :BPC * 128],
                                     mybir.ActivationFunctionType.Sigmoid,
                                     bias=b2t[:])
                nc.gpsimd.dma_start(out_ext[p], ot[:])

    nc.compile()
    return nc


def prepare_inputs(X, edge_index, W1, b1, W2, b2):
    """Host-side graph/layout prep. Returns per-core in_maps."""
    X = np.asarray(X, dtype=np.float32)
    edge_index = np.asarray(edge_index)
    W1 = np.asarray(W1, dtype=np.float32)
    b1 = np.asarray(b1, dtype=np.float32)
    W2 = np.asarray(W2, dtype=np.float32)
    b2 = np.asarray(b2, dtype=np.float32)

    src = edge_index[0].astype(np.int64)
    dst = edge_index[1].astype(np.int64)

    deg = np.bincount(dst, minlength=N).astype(np.float32) + 1.0
    dinv = 1.0 / np.sqrt(deg)
    dinv_pad = np.zeros(NP, np.float32)
    dinv_pad[:N] = dinv

    # M = Adj + I with multiplicity, uint8 counts
    Mfull = np.zeros((NP, NP), np.uint8)
    np.add.at(Mfull, (dst, src), 1)
    Mfull[np.arange(N), np.arange(N)] += 1
    assert Mfull.max() <= 15, "fp8e4 exact-int range exceeded"

    # XB: [NB, 128=(h,cin), PAIRS*128] with dinv-src folded in; s = 2*pl+h
    Xs = X * dinv[None, :, None, None]                  # [B, N, T, C]
    XT = np.zeros((S, C, NP), np.float32)
    XT[:, :, :N] = np.transpose(Xs, (0, 2, 3, 1)).reshape(S, C, N)
    x6 = XT.reshape(PAIRS, 2, C, NB, 128)
    XB = np.ascontiguousarray(np.transpose(x6, (3, 1, 2, 0, 4)))
    XB = XB.reshape(NB, 128, PAIRS * 128).astype(ml_dtypes.bfloat16)

    def blockdiag(W):
        D = np.zeros((128, 128), np.float32)
        D[:64, :64] = W
        D[64:, 64:] = W
        return D.astype(ml_dtypes.bfloat16)

    W1d = blockdiag(W1)
    W2d = blockdiag(W2)
    B1 = np.tile(b1, (128, F // C)).astype(np.float32)
    B2 = np.concatenate([b2, b2])[:, None].astype(np.float32)

    in_maps = []
    for c in range(N_CORES):
        rows = Mfull[c * BPC * 128:(c + 1) * BPC * 128, :]
        Mc = rows.reshape(BPC, 128, NB, 128).transpose(0, 3, 2, 1)
        Mc = np.ascontiguousarray(Mc).reshape(BPC, 128, NB * 128)
        Mc = Mc.astype(ml_dtypes.float8_e4m3)
        DI = dinv_pad[c * BPC * 128:(c + 1) * BPC * 128]
        DI = DI.reshape(BPC, 128).T.astype(np.float32)
        DI = np.ascontiguousarray(DI)
        in_maps.append({"XB": XB, "M": Mc, "W1d": W1d, "W2d": W2d,
                        "B1": B1, "B2": B2, "DI": DI})
    return in_maps


_NC_CACHE = {}


def kernel(X, edge_index, W1, b1, W2, b2):
    if "nc" not in _NC_CACHE:
        _NC_CACHE["nc"] = build_program(with_collective=True)
    nc = _NC_CACHE["nc"]
    in_maps = prepare_inputs(X, edge_index, W1, b1, W2, b2)

    res = None
    for attempt in range(5):
        try:
            res = run_bass_kernel_spmd(nc, in_maps, list(range(N_CORES)))
            break
        except Exception:
            if attempt == 4:
                raise
            time.sleep(60.0 * (attempt + 1))
    assert res is not None

    # reassemble: per core [12, 128, 1280] -> [24, 64, 1280]
    full = np.zeros((S, C, N), np.float32)
    for c in range(N_CORES):
        o = res.results[c]["OUT"].reshape(S, C, BPC * 128)
        lo = c * BPC * 128
        hi = min(N, (c + 1) * BPC * 128)
        if lo < N:
            full[:, :, lo:hi] = o[:, :, :hi - lo]
    out = full.reshape(B, T, C, N).transpose(0, 3, 1, 2)
    return np.ascontiguousarray(out)
